# revision 1
# baseline (speedup 1.0000x reference)
"""Causal self-attention (B=4, T=2048, d_model=d_k=1024, fp32) on 8 TRN2 cores.

Sharding: core c -> (batch b = c//2, parity par = c%2). Each core handles the
8 query blocks {par, par+2, ..., par+14} (block-cyclic over the 16 blocks of
128 rows), which balances causal work exactly across the pair. Each core
computes Q for its rows and K/V for the whole batch on-chip (no collectives).

On-device pipeline (all matmuls fp32r: ~1 cycle/row, ~1e-4 rounding):
  xT (d_model-major) staged in SBUF -> Q^T [dk, q] resident; keys streamed in
  quarters (512 keys): K^T projected per quarter, scores computed transposed
  (S^T[k, q]) over the contiguous window of active superslots, exp'd with the
  1/sqrt(dk) scale folded in, V projected while exp drains, then P^T@V and
  P^T@ones give O and the softmax denominators. O accumulated in SBUF across
  quarters; each superslot normalized and written out after its diagonal
  quarter. PE clock-gate (HAM) pre-warmed with dummy matmuls during the
  startup DMA preamble.
"""
import numpy as np

import concourse.bacc as bacc
import concourse.mybir as mybir
import concourse.tile as tile
from concourse.bass_utils import run_bass_kernel_spmd

F32 = mybir.dt.float32
F32R = mybir.dt.float32r
Exp = mybir.ActivationFunctionType.Exp

B, T, D, DK = 4, 2048, 1024, 1024
NCORES = 8
NSLOT = 8                # query blocks per core
NSS = 4                  # superslots of 256 query cols
NKE = 8                  # key eighths (256 keys each)
NEG = -1.0e9

_PROG_CACHE = {}


def _build_program():
    nc = bacc.Bacc("TRN2", target_bir_lowering=False, debug=False)
    # fp32r inputs: host pre-rounds to the 8-bit-exponent/11-bit-mantissa grid
    xqT = nc.declare_dram_parameter("xqT", [D, 1024], F32R, isOutput=False)
    xkT = nc.declare_dram_parameter("xkT", [D, T], F32R, isOutput=False)
    wq_d = nc.declare_dram_parameter("wq", [D, DK], F32R, isOutput=False)
    wk_d = nc.declare_dram_parameter("wk", [D, DK], F32R, isOutput=False)
    wv_d = nc.declare_dram_parameter("wv", [D, DK], F32R, isOutput=False)
    mask_d = nc.declare_dram_parameter("mask", [NSS, 128, 4, 256], F32, isOutput=False)
    o_d = nc.declare_dram_parameter("o", [1024, DK], F32, isOutput=True)

    xqT_r = xqT.rearrange("(c p) q -> p c q", p=128)
    xkT_r = xkT.rearrange("(c p) t -> p c t", p=128)
    wq_r = wq_d.rearrange("(c p) k -> p c k", p=128)
    wk_r = wk_d.rearrange("(c p) k -> p c k", p=128)
    wv_r = wv_d.rearrange("(c p) k -> p c k", p=128)

    with tile.TileContext(nc) as tc:
        with (
            tc.tile_pool(name="persist", bufs=1) as persist,
            tc.tile_pool(name="wkv", bufs=1) as wkv,
            tc.tile_pool(name="ps_small", bufs=4, space="PSUM") as pp_small,
            tc.tile_pool(name="ps_o", bufs=2, space="PSUM") as pp_o,
        ):
            qt = persist.tile([128, 8, 1024], F32R)      # Q^T: [dk_in_chunk, dk_chunk, q]
            den_acc = persist.tile([128, NSS, 2, 2], F32)
            ones_f = persist.tile([128, 2], F32)
            ones_r = persist.tile([128, 2], F32R)
            nc.vector.memset(ones_f[:], 1.0)
            nc.vector.tensor_copy(out=ones_r[:], in_=ones_f[:])
            wk = wkv.tile([128, 8, DK], F32R, tag="wk")
            wv = wkv.tile([128, 8, DK], F32R, tag="wv")

            # ---- Phase 1: Q^T projection (q streamed in halves of 512) ----
            xkp = tc.alloc_tile_pool(name="xk", bufs=1)
            xk_q0 = xkp.tile([128, 8, 512], F32R, tag="xk")
            with (
                tc.tile_pool(name="p1w", bufs=1) as p1w,
                tc.tile_pool(name="p1x", bufs=1) as p1x,
                tc.tile_pool(name="warm", bufs=1) as warm,
            ):
                # warm the PE clock gate (HAM) with dummy matmuls while the
                # first weight/activation DMAs are in flight — otherwise the
                # first ~3.4us of real matmuls run at half clock
                wz_f = warm.tile([128, 512], F32)
                nc.vector.memset(wz_f[:], 0.0)
                wz = warm.tile([128, 512], F32R)
                nc.vector.tensor_copy(out=wz[:], in_=wz_f[:])
                for _ in range(44):
                    wps = pp_small.tile([128, 512], F32, tag="small")
                    nc.tensor.matmul(wps[:, 0:256], wz[:, 0:128], wz[:, 0:256],
                                     start=True, stop=True)
                wq = p1w.tile([128, 8, DK], F32R)
                xq_h0 = p1x.tile([128, 8, 512], F32R, tag="xqh0")
                xq_h1 = p1x.tile([128, 8, 512], F32R, tag="xqh1")
                xq_hs = [xq_h0, xq_h1]
                # issue transfers in first-consumer order: wq+xq (Q projection),
                # then wk+xk quarter 0 (first K^T), then wv
                for d in range(8):
                    nc.sync.dma_start(out=wq[:, d, :], in_=wq_r[:, d, :])
                    nc.sync.dma_start(out=xq_hs[0][:, d, :], in_=xqT_r[:, d, 0:512])
                for d in range(8):
                    nc.sync.dma_start(out=xq_hs[1][:, d, :], in_=xqT_r[:, d, 512:1024])
                for d in range(8):
                    nc.sync.dma_start(out=wk[:, d, :], in_=wk_r[:, d, :])
                for d in range(8):
                    nc.sync.dma_start(out=xk_q0[:, d, :], in_=xkT_r[:, d, 0:512])
                for d in range(8):
                    nc.sync.dma_start(out=wv[:, d, :], in_=wv_r[:, d, :])
                for n in range(2):
                    xq_h = xq_hs[n]
                    for c in range(8):
                        psum = pp_small.tile([128, 512], F32, tag="small")
                        for d in range(8):
                            nc.tensor.matmul(
                                psum[:], wq[:, d, c * 128:(c + 1) * 128],
                                xq_h[:, d, :], start=(d == 0), stop=(d == 7))
                        nc.scalar.copy(out=qt[:, c, n * 512:(n + 1) * 512], in_=psum[:])

            # ---- Phase 2: stream key quarters; project K^T/V; attention ----
            with (
                tc.tile_pool(name="oaccp", bufs=1) as oaccp,
                tc.tile_pool(name="kt", bufs=1) as ktp,
                tc.tile_pool(name="vt", bufs=1) as vtp,
                tc.tile_pool(name="pmask", bufs=2) as pmask,
                tc.tile_pool(name="pexp", bufs=1) as pexp,
                tc.tile_pool(name="fin", bufs=1) as fin,
            ):
                o_acc = oaccp.tile([128, NSS, 2, 1024], F32)
                for u in range(NSS):        # key quarter: keys [512u, 512u+512)
                    act = NSS - u           # active superslots (contiguous window)
                    if u == 0:
                        xk_q = xk_q0        # preloaded during phase 1
                    else:
                        xk_q = xkp.tile([128, 8, 512], F32R, tag="xk")
                        for d in range(8):
                            nc.sync.dma_start(
                                out=xk_q[:, d, :], in_=xkT_r[:, d, u * 512:(u + 1) * 512])
                    # K^T for this quarter: [dk_in_chunk, dk_chunk, 512 keys]
                    kt_q = ktp.tile([128, 8, 512], F32R, tag="kt")
                    for c in range(8):
                        psum = pp_small.tile([128, 512], F32, tag="small")
                        for d in range(8):
                            nc.tensor.matmul(
                                psum[:], wk[:, d, c * 128:(c + 1) * 128],
                                xk_q[:, d, :], start=(d == 0), stop=(d == 7))
                        nc.scalar.copy(out=kt_q[:, c, :], in_=psum[:])
                    # causal mask for the diagonal superslot u, this quarter
                    m_sb = pmask.tile([128, 4, 256], F32, tag="mask")
                    nc.sync.dma_start(out=m_sb[:], in_=mask_d[u, :, :, :])

                    # scores S^T over the whole active window, exp'd
                    p_sb = pexp.tile([128, 4, 1024], F32R, tag="p")
                    for kb in range(4):
                        nch = (act * 256 + 511) // 512
                        ps0 = pp_small.tile([128, 512], F32, tag="small")
                        if nch > 1:
                            ps1 = pp_small.tile([128, 512], F32, tag="small")
                        else:
                            ps1 = None
                        w1 = 512
                        w1sz = act * 256 - 512 if nch > 1 else 0
                        w0sz = min(512, act * 256)
                        for c in range(8):
                            lhsT = kt_q[:, c, kb * 128:(kb + 1) * 128]
                            nc.tensor.matmul(
                                ps0[:, :w0sz], lhsT,
                                qt[:, c, u * 256: u * 256 + w0sz],
                                start=(c == 0), stop=(c == 7))
                            if ps1 is not None:
                                nc.tensor.matmul(
                                    ps1[:, :w1sz], lhsT,
                                    qt[:, c, u * 256 + w1: u * 256 + w1 + w1sz],
                                    start=(c == 0), stop=(c == 7))
                        nc.vector.tensor_add(ps0[:, :256], ps0[:, :256], m_sb[:, kb, :])
                        nc.scalar.activation(
                            out=p_sb[:, kb, 0:w0sz], in_=ps0[:, :w0sz],
                            func=Exp, scale=1.0 / 32.0)
                        if ps1 is not None:
                            nc.scalar.activation(
                                out=p_sb[:, kb, w1:w1 + w1sz], in_=ps1[:, :w1sz],
                                func=Exp, scale=1.0 / 32.0)

                    # V for this quarter: [key_in_block, kb, dk]
                    v_q = vtp.tile([128, 4, 1024], F32R, tag="v")
                    for kb in range(4):
                        for nn in range(2):
                            psum = pp_small.tile([128, 512], F32, tag="small")
                            for d in range(8):
                                nc.tensor.matmul(
                                    psum[:], xk_q[:, d, kb * 128:(kb + 1) * 128],
                                    wv[:, d, nn * 512:(nn + 1) * 512],
                                    start=(d == 0), stop=(d == 7))
                            nc.vector.tensor_copy(
                                out=v_q[:, kb, nn * 512:(nn + 1) * 512], in_=psum[:])

                    # O += P^T V ; den += P^T 1
                    for i in range(u, NSS):
                        for qc in range(2):
                            off = (i - u) * 256 + qc * 128
                            po = pp_o.tile([128, 1024], F32, tag="o")
                            den = pp_small.tile([128, 512], F32, tag="small")
                            for kb in range(4):
                                lhsT = p_sb[:, kb, off:off + 128]
                                for nn in range(2):
                                    nc.tensor.matmul(
                                        po[:, nn * 512:(nn + 1) * 512], lhsT,
                                        v_q[:, kb, nn * 512:(nn + 1) * 512],
                                        start=(kb == 0), stop=(kb == 3))
                                nc.tensor.matmul(
                                    den[:, :2], lhsT, ones_r[:],
                                    start=(kb == 0), stop=(kb == 3))
                            if u == 0:
                                nc.vector.tensor_copy(out=o_acc[:, i, qc, :], in_=po[:])
                                nc.vector.tensor_copy(out=den_acc[:, i, qc, :], in_=den[:, :2])
                            else:
                                nc.vector.tensor_add(
                                    o_acc[:, i, qc, :], o_acc[:, i, qc, :], po[:])
                                nc.vector.tensor_add(
                                    den_acc[:, i, qc, :], den_acc[:, i, qc, :], den[:, :2])
                            if i == u:
                                # superslot u complete: normalize + write out now
                                # (overlaps the remaining AV chains / next quarter)
                                rec = fin.tile([128, 1], F32, tag="rec")
                                nc.vector.reciprocal(out=rec[:], in_=den_acc[:, u, qc, 0:1])
                                outt = fin.tile([128, 1024], F32, tag="out")
                                nc.vector.tensor_scalar_mul(outt[:], o_acc[:, u, qc, :], rec[:])
                                s = 2 * u + qc
                                nc.sync.dma_start(out=o_d[s * 128:(s + 1) * 128, :], in_=outt[:])
            xkp.release()

    nc.finalize()
    return nc


def _masks(par: int) -> np.ndarray:
    """Additive causal masks, (NSS, 128, 4, 256) = [ss, key_in_blk, kblock, qcol];
    covers key blocks [4i, 4i+4) of superslot i (its diagonal quarter)."""
    m = np.zeros((NSS, 128, 4, 256), dtype=np.float32)
    p = np.arange(128)
    r = np.arange(256)
    slotq, rr = r // 128, r % 128
    for i in range(NSS):
        for kb in range(4):
            kglob = (4 * i + kb) * 128 + p                       # (128,)
            qglob = (4 * i + 2 * slotq + par) * 128 + rr          # (256,)
            m[i, :, kb, :] = np.where(kglob[:, None] <= qglob[None, :], 0.0, NEG)
    return np.ascontiguousarray(m)


def _round_fp32r(a: np.ndarray) -> np.ndarray:
    """Round-to-nearest-even onto the fp32r grid (top 20 bits of fp32)."""
    u = np.ascontiguousarray(a, dtype=np.float32).view(np.uint32)
    r = (u + np.uint32(0x7FF) + ((u >> np.uint32(12)) & np.uint32(1))) & np.uint32(0xFFFFF000)
    return r.view(np.float32)


def kernel(x: np.ndarray, Wq: np.ndarray, Wk: np.ndarray, Wv: np.ndarray) -> np.ndarray:
    x = np.ascontiguousarray(np.asarray(x, dtype=np.float32))
    Wq = _round_fp32r(np.asarray(Wq, dtype=np.float32))
    Wk = _round_fp32r(np.asarray(Wk, dtype=np.float32))
    Wv = _round_fp32r(np.asarray(Wv, dtype=np.float32))

    if "nc" not in _PROG_CACHE:
        _PROG_CACHE["nc"] = _build_program()
        _PROG_CACHE["masks"] = (_masks(0), _masks(1))
    nc = _PROG_CACHE["nc"]
    mask0, mask1 = _PROG_CACHE["masks"]

    in_maps = []
    slot_rows = []
    for c in range(NCORES):
        b, par = c // 2, c % 2
        blocks = [2 * s + par for s in range(NSLOT)]
        rows = np.concatenate([np.arange(p * 128, (p + 1) * 128) for p in blocks])
        slot_rows.append((b, rows))
        xT = _round_fp32r(np.ascontiguousarray(x[b].T))    # (D, T)
        xqT = np.ascontiguousarray(xT[:, rows])            # (D, 1024)
        in_maps.append({
            "xqT": xqT, "xkT": xT,
            "wq": Wq, "wk": Wk, "wv": Wv,
            "mask": mask1 if par else mask0,
        })
    _PROG_CACHE["last_in_maps"] = in_maps

    res = run_bass_kernel_spmd(nc, in_maps, core_ids=list(range(NCORES)))

    out = np.empty((B, T, DK), dtype=np.float32)
    for c in range(NCORES):
        b, rows = slot_rows[c]
        out[b, rows, :] = res.results[c]["o"]
    return out



# revision 4
# speedup vs baseline: 1.1315x; 1.1315x over previous
"""Causal self-attention (B=4, T=2048, d_model=d_k=1024, fp32) on 8 TRN2 cores.

Sharding: core c -> (batch b = c//2, parity par = c%2). Each core handles the
8 query blocks {par, par+2, ..., par+14} (block-cyclic over the 16 blocks of
128 rows), which balances causal work exactly across the pair.

Algebraic restructure (the big win vs the direct QKV pipeline): the host
feeds M = Wq @ Wk^T, so
  scores = Xq M Xk^T   -> A^T = proj(M, Xq^T) once (2.15 GF), then S^T
                          chains use raw Xk^T slices as stationary: the
                          K projection (4.3 GF/core) vanishes.
  O = P V = (P Xk) Wv  -> accumulate B^T[d,q] = sum_k Xk[k,d] P[q,k] per key
                          quarter (stationary = raw Xk in [k,d] layout,
                          2.68 GF), then one final O = B Wv projection per
                          query block (2.15 GF): the V projection (4.3
                          GF/core) vanishes.
Device matmul work per core: 9.66 GF vs 16.1 GF direct.

Softmax denominators via ones-stationary matmul passes over P^T (out [1,q]
row), transposed back to [q,1] partition layout at finalize time with a tiny
[1,128]-stationary matmul. All matmuls fp32r (~1e-4 rounding); PE clock-gate
(HAM) pre-warmed with dummy matmuls during the startup DMA preamble.
"""
import numpy as np

import concourse.bacc as bacc
import concourse.mybir as mybir
import concourse.tile as tile
from concourse.bass_utils import run_bass_kernel_spmd

F32 = mybir.dt.float32
F32R = mybir.dt.float32r
Exp = mybir.ActivationFunctionType.Exp

B, T, D, DK = 4, 2048, 1024, 1024
NCORES = 8
NSLOT = 8                # query blocks per core
NSS = 4                  # superslots of 256 query cols
NEG = -1.0e9

_PROG_CACHE = {}


def _build_program():
    nc = bacc.Bacc("TRN2", target_bir_lowering=False, debug=False)
    # fp32r inputs: host pre-rounds to the 8-bit-exponent/11-bit-mantissa grid
    xqT = nc.declare_dram_parameter("xqT", [D, 1024], F32R, isOutput=False)
    xkT = nc.declare_dram_parameter("xkT", [D, T], F32R, isOutput=False)
    xkd = nc.declare_dram_parameter("xkd", [T, D], F32R, isOutput=False)
    m_d = nc.declare_dram_parameter("m", [D, D], F32R, isOutput=False)
    wv_d = nc.declare_dram_parameter("wv", [D, DK], F32R, isOutput=False)
    mask_d = nc.declare_dram_parameter("mask", [NSS, 128, 4, 256], F32, isOutput=False)
    o_d = nc.declare_dram_parameter("o", [1024, DK], F32, isOutput=True)

    xqT_r = xqT.rearrange("(c p) q -> p c q", p=128)
    xkT_r = xkT.rearrange("(c p) t -> p c t", p=128)
    xkd_r = xkd.rearrange("(kb p) d -> p kb d", p=128)
    m_r = m_d.rearrange("(c p) k -> p c k", p=128)
    wv_r = wv_d.rearrange("(c p) k -> p c k", p=128)

    with tile.TileContext(nc) as tc:
        with (
            tc.tile_pool(name="persist", bufs=1) as persist,
            tc.tile_pool(name="wvp", bufs=1) as wvp,
            tc.tile_pool(name="ps_small", bufs=4, space="PSUM") as pp_small,
            tc.tile_pool(name="ps_b", bufs=3, space="PSUM") as pp_b,
        ):
            at = persist.tile([128, 8, 1024], F32R)      # A^T: [d_in_chunk, d_chunk, q]
            bt = persist.tile([128, 8, 1024], F32R)      # B^T: [d_in_chunk, d_chunk, q]
            den_row = persist.tile([1, 1024], F32)       # softmax denominators [1, q]
            ones_f = persist.tile([128, 2], F32)
            ones_r = persist.tile([128, 2], F32R)
            nc.vector.memset(ones_f[:], 1.0)
            nc.vector.tensor_copy(out=ones_r[:], in_=ones_f[:])
            wv = wvp.tile([128, 8, DK], F32R, tag="wv")

            # ---- Phase 1: A^T = (Xq M)^T projection (q streamed in halves) ----
            xkp = tc.alloc_tile_pool(name="xk", bufs=1)
            xk_q0 = xkp.tile([128, 8, 512], F32R, tag="xk")
            xkd_q0 = xkp.tile([128, 4, 1024], F32R, tag="xkd")
            with (
                tc.tile_pool(name="p1m", bufs=1) as p1m,
                tc.tile_pool(name="p1x", bufs=1) as p1x,
                tc.tile_pool(name="warm", bufs=1) as warm,
            ):
                # warm the PE clock gate (HAM) with dummy matmuls while the
                # first weight/activation DMAs are in flight — otherwise the
                # first ~3.4us of real matmuls run at half clock
                wz_f = warm.tile([128, 512], F32)
                nc.vector.memset(wz_f[:], 0.0)
                wz = warm.tile([128, 512], F32R)
                nc.vector.tensor_copy(out=wz[:], in_=wz_f[:])
                for _ in range(24):
                    wps = pp_small.tile([128, 512], F32, tag="small")
                    nc.tensor.matmul(wps[:, 0:256], wz[:, 0:128], wz[:, 0:256],
                                     start=True, stop=True)
                m_t = p1m.tile([128, 8, 1024], F32R)
                xq_h0 = p1x.tile([128, 8, 512], F32R, tag="xqh0")
                xq_h1 = p1x.tile([128, 8, 512], F32R, tag="xqh1")
                xq_hs = [xq_h0, xq_h1]
                # transfers in first-consumer order: m chunk c0 + xq half 0
                # (first A^T chain), then the rest of m, xq half 1, then the
                # quarter-0 key tiles
                for d in range(8):
                    nc.sync.dma_start(out=m_t[:, d, 0:128], in_=m_r[:, d, 0:128])
                    nc.sync.dma_start(out=xq_h0[:, d, :], in_=xqT_r[:, d, 0:512])
                for c in range(1, 8):
                    for d in range(8):
                        nc.sync.dma_start(out=m_t[:, d, c * 128:(c + 1) * 128],
                                          in_=m_r[:, d, c * 128:(c + 1) * 128])
                for d in range(8):
                    nc.sync.dma_start(out=xq_h1[:, d, :], in_=xqT_r[:, d, 512:1024])
                for d in range(8):
                    nc.sync.dma_start(out=xk_q0[:, d, :], in_=xkT_r[:, d, 0:512])
                for kb in range(4):
                    nc.sync.dma_start(out=xkd_q0[:, kb, :], in_=xkd_r[:, kb, :])
                for n in range(2):
                    xq_h = xq_hs[n]
                    for c in range(8):
                        psum = pp_small.tile([128, 512], F32, tag="small")
                        for d in range(8):
                            nc.tensor.matmul(
                                psum[:], m_t[:, d, c * 128:(c + 1) * 128],
                                xq_h[:, d, :], start=(d == 0), stop=(d == 7))
                        nc.scalar.copy(out=at[:, c, n * 512:(n + 1) * 512], in_=psum[:])

            # ---- Phase 2: stream key quarters; S^T, exp, den, B^T ----
            with (
                tc.tile_pool(name="pmask", bufs=2) as pmask,
                tc.tile_pool(name="pexp", bufs=1) as pexp,
            ):
                for u in range(NSS):        # key quarter: keys [512u, 512u+512)
                    act = NSS - u           # active superslots (contiguous window)
                    if u == 0:
                        xk_q = xk_q0        # preloaded during phase 1
                        xkd_q = xkd_q0
                    else:
                        xk_q = xkp.tile([128, 8, 512], F32R, tag="xk")
                        for d in range(8):
                            nc.sync.dma_start(
                                out=xk_q[:, d, :], in_=xkT_r[:, d, u * 512:(u + 1) * 512])
                        xkd_q = xkp.tile([128, 4, 1024], F32R, tag="xkd")
                        for kb in range(4):
                            nc.sync.dma_start(
                                out=xkd_q[:, kb, :], in_=xkd_r[:, u * 4 + kb, :])
                    if u == 2:
                        # wv only needed for the final O projection; stream it
                        # in behind the quarter-2/3 tiles
                        for d in range(8):
                            nc.sync.dma_start(out=wv[:, d, :], in_=wv_r[:, d, :])
                    # causal mask for the diagonal superslot u, this quarter
                    m_sb = pmask.tile([128, 4, 256], F32, tag="mask")
                    nc.sync.dma_start(out=m_sb[:], in_=mask_d[u, :, :, :])

                    # window chunks of <=512 query cols
                    chunks = []
                    o = 0
                    while o < act * 256:
                        cw = min(512, act * 256 - o)
                        chunks.append((o, cw))
                        o += cw

                    # scores S^T over the whole active window, exp'd
                    # stationary = raw Xk^T slices (no K projection!)
                    p_sb = pexp.tile([128, 4, 1024], F32R, tag="p")
                    for kb in range(4):
                        pss = [pp_small.tile([128, 512], F32, tag="small",
                                             name=f"ps_{u}_{kb}_{ci}")
                               for ci in range(len(chunks))]
                        for c in range(8):
                            lhsT = xk_q[:, c, kb * 128:(kb + 1) * 128]
                            for (o, cw), ps in zip(chunks, pss):
                                nc.tensor.matmul(
                                    ps[:, :cw], lhsT,
                                    at[:, c, u * 256 + o: u * 256 + o + cw],
                                    start=(c == 0), stop=(c == 7))
                        nc.vector.tensor_add(pss[0][:, :256], pss[0][:, :256],
                                             m_sb[:, kb, :])
                        for (o, cw), ps in zip(chunks, pss):
                            nc.scalar.activation(
                                out=p_sb[:, kb, o:o + cw], in_=ps[:, :cw],
                                func=Exp, scale=1.0 / 32.0)

                    # B^T += Xk_quarter^T P^T ; stationary = raw Xk [k,d] slices
                    for (o, cw) in chunks:
                        for c in range(8):
                            psb = pp_b.tile([128, 512], F32, tag="b")
                            for kb in range(4):
                                nc.tensor.matmul(
                                    psb[:, :cw], xkd_q[:, kb, c * 128:(c + 1) * 128],
                                    p_sb[:, kb, o:o + cw],
                                    start=(kb == 0), stop=(kb == 3))
                            dst = bt[:, c, u * 256 + o: u * 256 + o + cw]
                            if u == 0:
                                nc.vector.tensor_copy(out=dst, in_=psb[:, :cw])
                            else:
                                nc.vector.tensor_add(dst, dst, psb[:, :cw])

                    # denominators: ones-stationary pass over P^T -> [1, q] row
                    for (o, cw) in chunks:
                        psd = pp_small.tile([128, 512], F32, tag="small")
                        for kb in range(4):
                            nc.tensor.matmul(
                                psd[0:1, :cw], ones_r[:, 0:1],
                                p_sb[:, kb, o:o + cw],
                                start=(kb == 0), stop=(kb == 3))
                        dst = den_row[0:1, u * 256 + o: u * 256 + o + cw]
                        if u == 0:
                            nc.vector.tensor_copy(out=dst, in_=psd[0:1, :cw])
                        else:
                            nc.vector.tensor_add(dst, dst, psd[0:1, :cw])
            xkp.release()

            # ---- Phase 3: O = B Wv per query block; normalize; write out ----
            with tc.tile_pool(name="fin", bufs=2) as fin:
                for s in range(8):
                    # denominator transpose [1,128] -> [128,1] via tiny
                    # SBUF->SBUF DMA (512 B, partition scatter)
                    dcol = fin.tile([128, 1], F32, tag="dcol")
                    nc.sync.dma_start(out=dcol[:, 0:1],
                                      in_=den_row[0:1, s * 128:(s + 1) * 128])
                    rec = fin.tile([128, 1], F32, tag="rec")
                    nc.vector.reciprocal(out=rec[:], in_=dcol[:, 0:1])
                    po0 = pp_b.tile([128, 512], F32, tag="b")
                    po1 = pp_b.tile([128, 512], F32, tag="b")
                    for nn, po in ((0, po0), (1, po1)):
                        for d in range(8):
                            nc.tensor.matmul(
                                po[:], bt[:, d, s * 128:(s + 1) * 128],
                                wv[:, d, nn * 512:(nn + 1) * 512],
                                start=(d == 0), stop=(d == 7))
                    outt = fin.tile([128, 1024], F32, tag="out")
                    nc.vector.tensor_scalar_mul(outt[:, 0:512], po0[:], rec[:])
                    nc.vector.tensor_scalar_mul(outt[:, 512:1024], po1[:], rec[:])
                    nc.sync.dma_start(out=o_d[s * 128:(s + 1) * 128, :], in_=outt[:])

    nc.finalize()
    return nc


def _masks(par: int) -> np.ndarray:
    """Additive causal masks, (NSS, 128, 4, 256) = [ss, key_in_blk, kblock, qcol];
    covers key blocks [4i, 4i+4) of superslot i (its diagonal quarter)."""
    m = np.zeros((NSS, 128, 4, 256), dtype=np.float32)
    p = np.arange(128)
    r = np.arange(256)
    slotq, rr = r // 128, r % 128
    for i in range(NSS):
        for kb in range(4):
            kglob = (4 * i + kb) * 128 + p                       # (128,)
            qglob = (4 * i + 2 * slotq + par) * 128 + rr          # (256,)
            m[i, :, kb, :] = np.where(kglob[:, None] <= qglob[None, :], 0.0, NEG)
    return np.ascontiguousarray(m)


def _round_fp32r(a: np.ndarray) -> np.ndarray:
    """Round-to-nearest-even onto the fp32r grid (top 20 bits of fp32)."""
    u = np.ascontiguousarray(a, dtype=np.float32).view(np.uint32)
    r = (u + np.uint32(0x7FF) + ((u >> np.uint32(12)) & np.uint32(1))) & np.uint32(0xFFFFF000)
    return r.view(np.float32)


def kernel(x: np.ndarray, Wq: np.ndarray, Wk: np.ndarray, Wv: np.ndarray) -> np.ndarray:
    x = np.ascontiguousarray(np.asarray(x, dtype=np.float32))
    Wq = np.asarray(Wq, dtype=np.float32)
    Wk = np.asarray(Wk, dtype=np.float32)
    M = _round_fp32r(Wq @ Wk.T)
    Wv = _round_fp32r(np.asarray(Wv, dtype=np.float32))

    if "nc" not in _PROG_CACHE:
        _PROG_CACHE["nc"] = _build_program()
        _PROG_CACHE["masks"] = (_masks(0), _masks(1))
    nc = _PROG_CACHE["nc"]
    mask0, mask1 = _PROG_CACHE["masks"]

    in_maps = []
    slot_rows = []
    for c in range(NCORES):
        b, par = c // 2, c % 2
        blocks = [2 * s + par for s in range(NSLOT)]
        rows = np.concatenate([np.arange(p * 128, (p + 1) * 128) for p in blocks])
        slot_rows.append((b, rows))
        xb = _round_fp32r(x[b])                            # (T, D)
        xT = np.ascontiguousarray(xb.T)                    # (D, T)
        xqT = np.ascontiguousarray(xT[:, rows])            # (D, 1024)
        in_maps.append({
            "xqT": xqT, "xkT": xT, "xkd": xb,
            "m": M, "wv": Wv,
            "mask": mask1 if par else mask0,
        })
    _PROG_CACHE["last_in_maps"] = in_maps

    res = run_bass_kernel_spmd(nc, in_maps, core_ids=list(range(NCORES)))

    out = np.empty((B, T, DK), dtype=np.float32)
    for c in range(NCORES):
        b, rows = slot_rows[c]
        out[b, rows, :] = res.results[c]["o"]
    return out


# revision 6
# speedup vs baseline: 1.5463x; 1.3666x over previous
"""Causal self-attention (B=4, T=2048, d_model=d_k=1024, fp32) on 8 TRN2 cores.

Sharding: core c -> (batch b = c//2, parity par = c%2). Each core handles the
8 query blocks {par, par+2, ..., par+14} (block-cyclic over the 16 blocks of
128 rows), which balances causal work exactly across the pair.

Algebraic restructure (the big win vs the direct QKV pipeline): the host
feeds M = Wq @ Wk^T, so
  scores = Xq M Xk^T   -> A^T = proj(M, Xq^T) once (2.15 GF), then S^T
                          chains use raw Xk^T slices as stationary: the
                          K projection (4.3 GF/core) vanishes.
  O = P V = (P Xk) Wv  -> accumulate B^T[d,q] = sum_k Xk[k,d] P[q,k] per key
                          quarter (stationary = raw Xk in [k,d] layout,
                          2.68 GF), then one final O = B Wv projection per
                          query block (2.15 GF): the V projection (4.3
                          GF/core) vanishes.
Device matmul work per core: 9.66 GF vs 16.1 GF direct.

Softmax denominators via ones-stationary matmul passes over P^T (out [1,q]
row), transposed back to [q,1] partition layout at finalize time with a tiny
[1,128]-stationary matmul. All matmuls fp32r (~1e-4 rounding); PE clock-gate
(HAM) pre-warmed with dummy matmuls during the startup DMA preamble.
"""
import numpy as np

import concourse.bacc as bacc
import concourse.mybir as mybir
import concourse.tile as tile
from concourse.bass_utils import run_bass_kernel_spmd

F32 = mybir.dt.float32
F32R = mybir.dt.float32r
Exp = mybir.ActivationFunctionType.Exp

B, T, D, DK = 4, 2048, 1024, 1024
NCORES = 8
NSLOT = 8                # query blocks per core
NSS = 4                  # superslots of 256 query cols
NEG = -1.0e9

_PROG_CACHE = {}


def _build_program():
    nc = bacc.Bacc("TRN2", target_bir_lowering=False, debug=False)
    # fp32r inputs: host pre-rounds to the 8-bit-exponent/11-bit-mantissa grid
    xqT = nc.declare_dram_parameter("xqT", [D, 1024], F32R, isOutput=False)
    xkT = nc.declare_dram_parameter("xkT", [D, T], F32R, isOutput=False)
    xkd = nc.declare_dram_parameter("xkd", [T, D], F32R, isOutput=False)
    m_d = nc.declare_dram_parameter("m", [D, D], F32R, isOutput=False)
    wv_d = nc.declare_dram_parameter("wv", [D, DK], F32R, isOutput=False)
    mask_d = nc.declare_dram_parameter("mask", [NSS, 128, 4, 256], F32, isOutput=False)
    o_d = nc.declare_dram_parameter("o", [1024, DK], F32, isOutput=True)

    xqT_r = xqT.rearrange("(c p) q -> p c q", p=128)
    xkT_r = xkT.rearrange("(c p) t -> p c t", p=128)
    xkd_r = xkd.rearrange("(kb p) d -> p kb d", p=128)
    m_r = m_d.rearrange("(c p) k -> p c k", p=128)
    wv_r = wv_d.rearrange("(c p) k -> p c k", p=128)

    with tile.TileContext(nc) as tc:
        with (
            tc.tile_pool(name="persist", bufs=1) as persist,
            tc.tile_pool(name="wvp", bufs=1) as wvp,
            tc.tile_pool(name="ps_small", bufs=4, space="PSUM") as pp_small,
            tc.tile_pool(name="ps_b", bufs=3, space="PSUM") as pp_b,
        ):
            at = persist.tile([128, 8, 1024], F32R)      # A^T: [d_in_chunk, d_chunk, q]
            bt = persist.tile([128, 8, 1024], F32R)      # B^T: [d_in_chunk, d_chunk, q]
            den_row = persist.tile([1, 1024], F32)       # softmax denominators [1, q]
            ones_f = persist.tile([128, 2], F32)
            ones_r = persist.tile([128, 2], F32R)
            nc.vector.memset(ones_f[:], 1.0)
            nc.vector.tensor_copy(out=ones_r[:], in_=ones_f[:])
            wv = wvp.tile([128, 8, DK], F32R, tag="wv")

            # ---- Phase 1: A^T = (Xq M)^T projection (q streamed in halves) ----
            xkp = tc.alloc_tile_pool(name="xk", bufs=2)
            xk_q0 = xkp.tile([128, 8, 512], F32R, tag="xk", name="xk_q0")
            with (
                tc.tile_pool(name="p1m", bufs=1) as p1m,
                tc.tile_pool(name="p1x", bufs=1) as p1x,
                tc.tile_pool(name="warm", bufs=1) as warm,
            ):
                # warm the PE clock gate (HAM) with dummy matmuls while the
                # first weight/activation DMAs are in flight — otherwise the
                # first ~3.4us of real matmuls run at half clock, and a long
                # dense burst here helps HAM latch the full clock for the
                # rest of the kernel
                wz_f = warm.tile([128, 512], F32)
                nc.vector.memset(wz_f[:], 0.0)
                wz = warm.tile([128, 512], F32R)
                nc.vector.tensor_copy(out=wz[:], in_=wz_f[:])
                for _ in range(44):
                    wps = pp_small.tile([128, 512], F32, tag="small")
                    nc.tensor.matmul(wps[:, 0:256], wz[:, 0:128], wz[:, 0:256],
                                     start=True, stop=True)
                m_t = p1m.tile([128, 8, 1024], F32R)
                xq_h0 = p1x.tile([128, 8, 512], F32R, tag="xqh0")
                xq_h1 = p1x.tile([128, 8, 512], F32R, tag="xqh1")
                xq_hs = [xq_h0, xq_h1]
                # transfers in first-consumer order: m + xq half 0 (first A^T
                # chains), then xq half 1, then the quarter-0 key tile
                for d in range(8):
                    nc.sync.dma_start(out=m_t[:, d, :], in_=m_r[:, d, :])
                    nc.sync.dma_start(out=xq_h0[:, d, :], in_=xqT_r[:, d, 0:512])
                for d in range(8):
                    nc.sync.dma_start(out=xq_h1[:, d, :], in_=xqT_r[:, d, 512:1024])
                for d in range(8):
                    nc.sync.dma_start(out=xk_q0[:, d, :], in_=xkT_r[:, d, 0:512])
                for n in range(2):
                    xq_h = xq_hs[n]
                    for c in range(8):
                        psum = pp_small.tile([128, 512], F32, tag="small")
                        for d in range(8):
                            nc.tensor.matmul(
                                psum[:], m_t[:, d, c * 128:(c + 1) * 128],
                                xq_h[:, d, :], start=(d == 0), stop=(d == 7))
                        nc.scalar.copy(out=at[:, c, n * 512:(n + 1) * 512], in_=psum[:])

            # ---- Phase 2: stream key quarters; S^T + exp for quarter u while
            # B^T/den for quarter u-1 runs (software pipeline keeps the PE
            # gapless across the exp handoff) ----
            with (
                tc.tile_pool(name="xkdp", bufs=2) as xkdp,
                tc.tile_pool(name="pmask", bufs=2) as pmask,
                tc.tile_pool(name="pexp", bufs=2) as pexp,
            ):
                def win_chunks(act):
                    chunks = []
                    o = 0
                    while o < act * 256:
                        cw = min(512, act * 256 - o)
                        chunks.append((o, cw))
                        o += cw
                    return chunks

                def emit_bt_den(u, chunks, p_sb, xkd_q):
                    # B^T += Xk_quarter^T P^T; stationary = raw Xk [k,d] slices
                    for (o, cw) in chunks:
                        for c in range(8):
                            psb = pp_b.tile([128, 512], F32, tag="b",
                                            name=f"psb_{u}_{o}_{c}")
                            for kb in range(4):
                                nc.tensor.matmul(
                                    psb[:, :cw], xkd_q[:, kb, c * 128:(c + 1) * 128],
                                    p_sb[:, kb, o:o + cw],
                                    start=(kb == 0), stop=(kb == 3))
                            dst = bt[:, c, u * 256 + o: u * 256 + o + cw]
                            if u == 0:
                                nc.vector.tensor_copy(out=dst, in_=psb[:, :cw])
                            else:
                                nc.vector.tensor_add(dst, dst, psb[:, :cw])
                    # denominators: ones-stationary pass over P^T -> [1, q] row
                    for (o, cw) in chunks:
                        psd = pp_small.tile([128, 512], F32, tag="small",
                                            name=f"psd_{u}_{o}")
                        for kb in range(4):
                            nc.tensor.matmul(
                                psd[0:1, :cw], ones_r[:, 0:1],
                                p_sb[:, kb, o:o + cw],
                                start=(kb == 0), stop=(kb == 3))
                        dst = den_row[0:1, u * 256 + o: u * 256 + o + cw]
                        if u == 0:
                            nc.vector.tensor_copy(out=dst, in_=psd[0:1, :cw])
                        else:
                            nc.vector.tensor_add(dst, dst, psd[0:1, :cw])

                xkd_q0 = xkdp.tile([128, 4, 1024], F32R, tag="xkd", name="xkd_q0")
                for kb in range(4):
                    nc.sync.dma_start(out=xkd_q0[:, kb, :], in_=xkd_r[:, kb, :])
                m_sb0 = pmask.tile([128, 4, 256], F32, tag="mask", name="m_sb0")
                nc.sync.dma_start(out=m_sb0[:], in_=mask_d[0, :, :, :])

                xk_qs, xkd_qs, m_sbs = {0: xk_q0}, {0: xkd_q0}, {0: m_sb0}
                prev = None
                for u in range(NSS):        # key quarter: keys [512u, 512u+512)
                    act = NSS - u           # active superslots (contiguous window)
                    if u + 1 < NSS:
                        # prefetch next quarter's tiles (land during this one)
                        un = u + 1
                        xk_n = xkp.tile([128, 8, 512], F32R, tag="xk",
                                        name=f"xk_q{un}")
                        for d in range(8):
                            nc.sync.dma_start(
                                out=xk_n[:, d, :],
                                in_=xkT_r[:, d, un * 512:(un + 1) * 512])
                        xkd_n = xkdp.tile([128, 4, 1024], F32R, tag="xkd",
                                          name=f"xkd_q{un}")
                        for kb in range(4):
                            nc.sync.dma_start(
                                out=xkd_n[:, kb, :], in_=xkd_r[:, un * 4 + kb, :])
                        m_n = pmask.tile([128, 4, 256], F32, tag="mask",
                                         name=f"m_sb{un}")
                        nc.sync.dma_start(out=m_n[:], in_=mask_d[un, :, :, :])
                        xk_qs[un], xkd_qs[un], m_sbs[un] = xk_n, xkd_n, m_n
                    if u == 1:
                        # wv only needed for the final O projection
                        for d in range(8):
                            nc.sync.dma_start(out=wv[:, d, :], in_=wv_r[:, d, :])

                    xk_q, m_sb = xk_qs[u], m_sbs[u]
                    chunks = win_chunks(act)
                    # scores S^T over the whole active window, exp'd
                    # stationary = raw Xk^T slices (no K projection!)
                    # chunk-outer c-loops keep consecutive LDWEIGHTS distinct
                    p_sb = pexp.tile([128, 4, 1024], F32R, tag="p",
                                     name=f"p_sb{u}")
                    for kb in range(4):
                        pss = [pp_small.tile([128, 512], F32, tag="small",
                                             name=f"ps_{u}_{kb}_{ci}")
                               for ci in range(len(chunks))]
                        for (o, cw), ps in zip(chunks, pss):
                            for c in range(8):
                                nc.tensor.matmul(
                                    ps[:, :cw],
                                    xk_q[:, c, kb * 128:(kb + 1) * 128],
                                    at[:, c, u * 256 + o: u * 256 + o + cw],
                                    start=(c == 0), stop=(c == 7))
                        nc.vector.tensor_add(pss[0][:, :256], pss[0][:, :256],
                                             m_sb[:, kb, :])
                        for (o, cw), ps in zip(chunks, pss):
                            nc.scalar.activation(
                                out=p_sb[:, kb, o:o + cw], in_=ps[:, :cw],
                                func=Exp, scale=1.0 / 32.0)

                    if prev is not None:
                        emit_bt_den(*prev)
                    prev = (u, chunks, p_sb, xkd_qs[u])
                emit_bt_den(*prev)
            xkp.release()

            # ---- Phase 3: O = B Wv per query block; normalize; write out ----
            with tc.tile_pool(name="fin", bufs=2) as fin:
                for s in range(8):
                    # denominator transpose [1,128] -> [128,1] via tiny
                    # SBUF->SBUF DMA (512 B, partition scatter)
                    dcol = fin.tile([128, 1], F32, tag="dcol")
                    nc.sync.dma_start(out=dcol[:, 0:1],
                                      in_=den_row[0:1, s * 128:(s + 1) * 128])
                    rec = fin.tile([128, 1], F32, tag="rec")
                    nc.vector.reciprocal(out=rec[:], in_=dcol[:, 0:1])
                    po0 = pp_b.tile([128, 512], F32, tag="b")
                    po1 = pp_b.tile([128, 512], F32, tag="b")
                    for nn, po in ((0, po0), (1, po1)):
                        for d in range(8):
                            nc.tensor.matmul(
                                po[:], bt[:, d, s * 128:(s + 1) * 128],
                                wv[:, d, nn * 512:(nn + 1) * 512],
                                start=(d == 0), stop=(d == 7))
                    outt = fin.tile([128, 1024], F32, tag="out")
                    nc.vector.tensor_scalar_mul(outt[:, 0:512], po0[:], rec[:])
                    nc.vector.tensor_scalar_mul(outt[:, 512:1024], po1[:], rec[:])
                    nc.sync.dma_start(out=o_d[s * 128:(s + 1) * 128, :], in_=outt[:])

    nc.finalize()
    return nc


def _masks(par: int) -> np.ndarray:
    """Additive causal masks, (NSS, 128, 4, 256) = [ss, key_in_blk, kblock, qcol];
    covers key blocks [4i, 4i+4) of superslot i (its diagonal quarter)."""
    m = np.zeros((NSS, 128, 4, 256), dtype=np.float32)
    p = np.arange(128)
    r = np.arange(256)
    slotq, rr = r // 128, r % 128
    for i in range(NSS):
        for kb in range(4):
            kglob = (4 * i + kb) * 128 + p                       # (128,)
            qglob = (4 * i + 2 * slotq + par) * 128 + rr          # (256,)
            m[i, :, kb, :] = np.where(kglob[:, None] <= qglob[None, :], 0.0, NEG)
    return np.ascontiguousarray(m)


def _round_fp32r(a: np.ndarray) -> np.ndarray:
    """Round-to-nearest-even onto the fp32r grid (top 20 bits of fp32)."""
    u = np.ascontiguousarray(a, dtype=np.float32).view(np.uint32)
    r = (u + np.uint32(0x7FF) + ((u >> np.uint32(12)) & np.uint32(1))) & np.uint32(0xFFFFF000)
    return r.view(np.float32)


def kernel(x: np.ndarray, Wq: np.ndarray, Wk: np.ndarray, Wv: np.ndarray) -> np.ndarray:
    x = np.ascontiguousarray(np.asarray(x, dtype=np.float32))
    Wq = np.asarray(Wq, dtype=np.float32)
    Wk = np.asarray(Wk, dtype=np.float32)
    M = _round_fp32r(Wq @ Wk.T)
    Wv = _round_fp32r(np.asarray(Wv, dtype=np.float32))

    if "nc" not in _PROG_CACHE:
        _PROG_CACHE["nc"] = _build_program()
        _PROG_CACHE["masks"] = (_masks(0), _masks(1))
    nc = _PROG_CACHE["nc"]
    mask0, mask1 = _PROG_CACHE["masks"]

    in_maps = []
    slot_rows = []
    for c in range(NCORES):
        b, par = c // 2, c % 2
        blocks = [2 * s + par for s in range(NSLOT)]
        rows = np.concatenate([np.arange(p * 128, (p + 1) * 128) for p in blocks])
        slot_rows.append((b, rows))
        xb = _round_fp32r(x[b])                            # (T, D)
        xT = np.ascontiguousarray(xb.T)                    # (D, T)
        xqT = np.ascontiguousarray(xT[:, rows])            # (D, 1024)
        in_maps.append({
            "xqT": xqT, "xkT": xT, "xkd": xb,
            "m": M, "wv": Wv,
            "mask": mask1 if par else mask0,
        })
    _PROG_CACHE["last_in_maps"] = in_maps

    res = run_bass_kernel_spmd(nc, in_maps, core_ids=list(range(NCORES)))

    out = np.empty((B, T, DK), dtype=np.float32)
    for c in range(NCORES):
        b, rows = slot_rows[c]
        out[b, rows, :] = res.results[c]["o"]
    return out
